# revision 1
# baseline (speedup 1.0000x reference)
"""Differential cross-attention Trainium2 kernel (8 NeuronCores).

Sharding: 8 cores = (batch b = c//2) x (head-pair group hg = c%2). Each
core computes 2 differential pairs (heads 2hg+p, 2hg+4+p for p in 0,1)
over the FULL 1024 queries and 1024 keys of its batch, producing a
partial output projection (its 256 model dims through Wp); the two
partials per batch are summed on the host. This avoids recomputing K/V
projections on two cores (vs. a query-split).

Per-core layout (128-partition tiles):
  Q_T/K_T [do, n] fp32 via PE (4 accumulation passes over DIM), score
  matmuls in float32r (1 cycle/row at N=512). V [k, dv] bf16 arranged
  per pair as [V2 | V1 | 1] so the PV stage needs exactly 2 bf16
  matmuls per (pair, ktile, qtile): U1 @ [V1|1] -> [P1V1, S1] and
  U2 @ [V2|V1|1] -> [P2V2, P2V1, S2], accumulating over ktiles in PSUM.
  exp on ACT ([k,q] tiles, no max subtraction; |scores| <= ~2), RPE
  bias exp(rpe) multiplied post-exp on DVE in bf16 (4x mode).
  Combine folds softmax + differential into per-q scalars applied with
  broadcast APs; PE transposes feed the output projection.
"""
import sys
sys.path.insert(0, "/opt/trn_rl_repo")
import numpy as np

DIM = 512
H = 8
HD = 64
NQ = 1024
NKV = 1024
MAX_DIST = 128
LAMBDA_INIT = 0.8
N_CORES = 8
SCALE = HD ** -0.5

_COMPILED = {}


def _build(reps=1, zb=True):
    import concourse.bacc as bacc
    import concourse.mybir as mybir
    from concourse.tile import TileContext
    from concourse.masks import make_identity

    f32 = mybir.dt.float32
    f32r = mybir.dt.float32r
    bf16 = mybir.dt.bfloat16
    ADD = mybir.AluOpType.add
    MUL = mybir.AluOpType.mult
    SUB = mybir.AluOpType.subtract
    EXP = mybir.ActivationFunctionType.Exp

    nc = bacc.Bacc("TRN2", target_bir_lowering=False, debug=False,
                   num_devices=N_CORES)

    xq = nc.dram_tensor("xq", [128, 4, NQ], bf16, kind="ExternalInput")
    xkv = nc.dram_tensor("xkv", [128, 4, NKV], bf16, kind="ExternalInput")
    wq = nc.dram_tensor("wq", [128, 4, 256], bf16, kind="ExternalInput")
    wk = nc.dram_tensor("wk", [128, 4, 256], bf16, kind="ExternalInput")
    wv = nc.dram_tensor("wv", [128, 4, 256], bf16, kind="ExternalInput")
    wp = nc.dram_tensor("wp", [128, 2, DIM], bf16, kind="ExternalInput")
    if not zb:
        bqk = nc.dram_tensor("bqk", [128, 4], f32, kind="ExternalInput")
        bv_d = nc.dram_tensor("bv", [128, 256], f32, kind="ExternalInput")
        bp_d = nc.dram_tensor("bp", [128, 4], f32, kind="ExternalInput")
    # packed per-q scalars: [:, 0:8] = 1+alpha, [:, 8:24] = alpha*lam[p]
    alp_d = nc.dram_tensor("alp", [128, 24, 1], f32, kind="ExternalInput")
    # [pair, qhalf, ktile, k, 2 heads * 512 q]
    biasT = nc.dram_tensor("biasT", [2, 2, 8, 128, NQ], bf16,
                           kind="ExternalInput")
    out_T = nc.dram_tensor("out_T", [DIM, NQ], bf16, kind="ExternalOutput")

    with TileContext(nc) as tc:
        with (
            tc.tile_pool(name="const", bufs=1) as cpool,
            tc.tile_pool(name="bias", bufs=6) as bpool,
            tc.tile_pool(name="u", bufs=3) as upool,
            tc.tile_pool(name="ub", bufs=3) as ubpool,
            tc.tile_pool(name="small", bufs=2) as rpool,
            tc.tile_pool(name="state", bufs=2) as stpool,
            tc.tile_pool(name="proj", bufs=2, space="PSUM") as ppool,
            tc.tile_pool(name="score", bufs=2, space="PSUM") as spool,
            tc.tile_pool(name="pvacc", bufs=1, space="PSUM") as pvpool,
        ):
            steps = [(p, qh, kt) for qh in range(2) for p in range(2)
                     for kt in range(8)]

            def make_rep():
                ctx = {}

                def head():
                    # tiles
                    wq_t = cpool.tile([128, 4, 256], bf16, tag="wq")
                    wk_t = cpool.tile([128, 4, 256], bf16, tag="wk")
                    wv_t = cpool.tile([128, 4, 256], bf16, tag="wv")
                    wp_t = cpool.tile([128, 2, DIM], bf16, tag="wp")
                    xq_t = stpool.tile([128, 4, NQ], bf16, tag="xq")
                    xkv_t = stpool.tile([128, 4, NKV], bf16, tag="xkv")
                    alp_t = cpool.tile([128, 24, 1], f32, tag="alp")
                    q_sb = stpool.tile([128, 2, NQ], f32r, tag="qsb")
                    k_sb = stpool.tile([128, 2, NKV], f32r, tag="ksb")
                    v_sb = stpool.tile([128, 8, 258], bf16, tag="vsb")
                    xcat = stpool.tile([128, 8, 256], bf16, tag="xcat")
                    xcat_T = stpool.tile([128, 2, NQ], bf16, tag="xcatT")
                    out_sb = stpool.tile([128, 4, NQ], bf16, tag="osb")
                    ident = cpool.tile([128, 128], bf16, tag="ident")
                    ctx.update(wq_t=wq_t, wk_t=wk_t, wv_t=wv_t, wp_t=wp_t,
                               xq_t=xq_t, xkv_t=xkv_t, alp_t=alp_t,
                               q_sb=q_sb, k_sb=k_sb, v_sb=v_sb, xcat=xcat,
                               xcat_T=xcat_T, out_sb=out_sb, ident=ident)
                    if not zb:
                        bqk_t = cpool.tile([128, 4], f32, tag="bqk")
                        bv_t = cpool.tile([128, 256], f32, tag="bv")
                        bp_t = cpool.tile([128, 4], f32, tag="bp")
                        ctx.update(bqk_t=bqk_t, bv_t=bv_t, bp_t=bp_t)
                        for t, s in ((bqk_t, bqk), (bv_t, bv_d), (bp_t, bp_d)):
                            nc.sync.dma_start(out=t[:], in_=s[:])
                    nc.sync.dma_start(out=alp_t[:], in_=alp_d[:])
                    nc.sync.dma_start(out=wq_t[:], in_=wq[:])
                    nc.sync.dma_start(out=xq_t[:, :, 0:512], in_=xq[:, :, 0:512])
                    nc.sync.dma_start(out=wk_t[:], in_=wk[:])
                    nc.sync.dma_start(out=xkv_t[:, :, 0:512], in_=xkv[:, :, 0:512])
                    nc.sync.dma_start(out=xkv_t[:, :, 512:1024],
                                      in_=xkv[:, :, 512:1024])
                    nc.sync.dma_start(out=xq_t[:, :, 512:1024],
                                      in_=xq[:, :, 512:1024])
                    nc.sync.dma_start(out=wv_t[:], in_=wv[:])
                    make_identity(nc, ident[:])
                    emit_q(0, 0)
                    emit_q(1, 0)
                    emit_k(0, 0)
                    emit_k(1, 0)
                    ctx["prework"] = (
                        [lambda: emit_k(0, 1), lambda: emit_k(1, 1)]
                        + [lambda kt=kt: emit_v(kt) for kt in range(3)]
                        + [lambda: emit_q(0, 1), lambda: emit_q(1, 1),
                           lambda: nc.sync.dma_start(out=ctx["wp_t"][:],
                                                     in_=wp[:])]
                        + [lambda kt=kt: emit_v(kt) for kt in range(3, 8)])

                def emit_q(t, qh):
                    ps = ppool.tile([128, 512], f32, tag="proj")
                    for c in range(4):
                        nc.tensor.matmul(
                            ps[:], lhsT=ctx["wq_t"][:, c, 128 * t:128 * (t + 1)],
                            rhs=ctx["xq_t"][:, c, 512 * qh:512 * (qh + 1)],
                            start=(c == 0), stop=(c == 3))
                    dst = ctx["q_sb"][:, t, 512 * qh:512 * (qh + 1)]
                    if zb:
                        nc.vector.tensor_copy(out=dst, in_=ps[:])
                    else:
                        nc.vector.tensor_scalar(
                            out=dst, in0=ps[:], scalar1=ctx["bqk_t"][:, t:t + 1],
                            scalar2=None, op0=ADD)

                def emit_k(t, qh):
                    ps = ppool.tile([128, 512], f32, tag="proj")
                    for c in range(4):
                        nc.tensor.matmul(
                            ps[:], lhsT=ctx["wk_t"][:, c, 128 * t:128 * (t + 1)],
                            rhs=ctx["xkv_t"][:, c, 512 * qh:512 * (qh + 1)],
                            start=(c == 0), stop=(c == 3))
                    dst = ctx["k_sb"][:, t, 512 * qh:512 * (qh + 1)]
                    if zb:
                        nc.vector.tensor_copy(out=dst, in_=ps[:])
                    else:
                        nc.vector.tensor_scalar(
                            out=dst, in0=ps[:],
                            scalar1=ctx["bqk_t"][:, 2 + t:3 + t],
                            scalar2=None, op0=ADD)

                def emit_v(kt):
                    ps = ppool.tile([128, 512], f32, tag="proj")
                    for c in range(4):
                        nc.tensor.matmul(
                            ps[:, 0:256],
                            lhsT=ctx["xkv_t"][:, c, 128 * kt:128 * (kt + 1)],
                            rhs=ctx["wv_t"][:, c, :],
                            start=(c == 0), stop=(c == 3))
                    v_sb = ctx["v_sb"]
                    for p in range(2):
                        if zb:
                            nc.vector.tensor_copy(
                                out=v_sb[:, kt, 129 * p:129 * p + 128],
                                in_=ps[:, 128 * p:128 * (p + 1)])
                        else:
                            nc.vector.tensor_tensor(
                                out=v_sb[:, kt, 129 * p:129 * p + 128],
                                in0=ps[:, 128 * p:128 * (p + 1)],
                                in1=ctx["bv_t"][:, 128 * p:128 * (p + 1)],
                                op=ADD)
                        nc.gpsimd.memset(
                            v_sb[:, kt, 129 * p + 128:129 * p + 129], 1.0)

                pv_cur = {}
                bias_cur = {}

                def emit_s(st):
                    p, qh, kt = st
                    ss = spool.tile([128, 1024], f32, tag="scores")
                    if kt % 4 == 0:
                        bt4 = bpool.tile([128, 4, 1024], bf16, tag="biasin")
                        nc.sync.dma_start(
                            out=bt4[:],
                            in_=biasT[p, qh, kt:kt + 4, :, :]
                            .rearrange("t p n -> p t n"))
                        bias_cur[(p, qh, kt // 4)] = bt4
                    bt = bias_cur[(p, qh, kt // 4)][:, kt % 4, :]
                    for j in range(2):
                        nc.tensor.matmul(
                            ss[:, 512 * j:512 * (j + 1)],
                            lhsT=ctx["k_sb"][64 * p:64 * (p + 1), j,
                                             128 * kt:128 * (kt + 1)],
                            rhs=ctx["q_sb"][64 * p:64 * (p + 1), j,
                                            512 * qh:512 * (qh + 1)],
                            start=True, stop=True)
                    u = upool.tile([128, 1024], bf16, tag="u")
                    nc.scalar.activation(u[:], ss[:], EXP)
                    ub = ubpool.tile([128, 1024], bf16, tag="ub")
                    # offload 1 in 4 bias multiplies to the idle Pool engine
                    eng = nc.gpsimd if kt in (2, 6) else nc.vector
                    eng.tensor_tensor(out=ub[:], in0=u[:], in1=bt, op=MUL)
                    return ub

                def emit_pv(st, ub):
                    p, qh, kt = st
                    if kt == 0:
                        pv_tile = pvpool.tile([128, 4, 256], f32, tag="pv")
                        pv_cur[(p, qh)] = pv_tile
                    pv = pv_cur[(p, qh)]
                    v_sb = ctx["v_sb"]
                    # start/stop are per-PSUM-bank epoch flags: one start
                    # (first matmul touching the bank zeroes it) and one stop
                    # (last of the epoch). Bank = qt//2 in this 2-bank tile.
                    for qt in range(4):
                        q0 = 128 * qt
                        nc.tensor.matmul(
                            pv[:, qt, 0:65], lhsT=ub[:, q0:q0 + 128],
                            rhs=v_sb[:, kt, 129 * p + 64:129 * p + 129],
                            start=(kt == 0 and qt % 2 == 0), stop=False)
                        nc.tensor.matmul(
                            pv[:, qt, 65:194],
                            lhsT=ub[:, 512 + q0:512 + q0 + 128],
                            rhs=v_sb[:, kt, 129 * p:129 * p + 129],
                            start=False, stop=(kt == 7 and qt % 2 == 1))

                def emit_combine(p, qh):
                    pv = pv_cur[(p, qh)]
                    al1_t = ctx["alp_t"][:, 0:8, :]
                    alam_t = ctx["alp_t"][:, 8:24, :]
                    xcat = ctx["xcat"]
                    rs1 = rpool.tile([128, 4, 1], f32, tag="rs1")
                    rs2 = rpool.tile([128, 4, 1], f32, tag="rs2")
                    g1 = rpool.tile([128, 4, 1], f32, tag="g1")
                    g2 = rpool.tile([128, 4, 1], f32, tag="g2")
                    nc.vector.reciprocal(rs1[:], pv[:, :, 64:65])
                    nc.vector.reciprocal(rs2[:], pv[:, :, 193:194])
                    nc.vector.tensor_tensor(
                        out=g1[:], in0=rs1[:],
                        in1=al1_t[:, 4 * qh:4 * qh + 4, :], op=MUL)
                    nc.vector.tensor_tensor(
                        out=g2[:], in0=rs2[:],
                        in1=alam_t[:, 8 * p + 4 * qh:8 * p + 4 * qh + 4, :],
                        op=MUL)
                    tmp1 = rpool.tile([128, 4, 64], f32, tag="tmp1")
                    tmp2 = rpool.tile([128, 4, 64], f32, tag="tmp2")
                    nc.vector.tensor_tensor(
                        out=tmp1[:], in0=pv[:, :, 0:64],
                        in1=g1[:].broadcast_to([128, 4, 64]), op=MUL)
                    nc.vector.tensor_tensor(
                        out=tmp2[:], in0=pv[:, :, 129:193],
                        in1=g2[:].broadcast_to([128, 4, 64]), op=MUL)
                    nc.vector.tensor_tensor(
                        out=xcat[:, 4 * qh:4 * qh + 4, 128 * p:128 * p + 64],
                        in0=tmp1[:], in1=tmp2[:], op=SUB)
                    nc.vector.tensor_tensor(
                        out=xcat[:, 4 * qh:4 * qh + 4,
                                 128 * p + 64:128 * (p + 1)],
                        in0=pv[:, :, 65:129],
                        in1=rs2[:].broadcast_to([128, 4, 64]), op=MUL)

                def emit_transpose_half(qh):
                    # 4 bf16 transposes packed per PSUM bank (per-bank epoch).
                    for dt in range(2):
                        tp = ppool.tile([128, 512], f32, tag="proj")
                        tpb = tp[:].bitcast(bf16)
                        for i in range(4):
                            qt = 4 * qh + i
                            nc.tensor.matmul(
                                tpb[:, 128 * i:128 * (i + 1)],
                                lhsT=ctx["xcat"][:, qt, 128 * dt:128 * (dt + 1)],
                                rhs=ctx["ident"][:], is_transpose=True,
                                start=(i == 0), stop=(i == 3))
                        nc.vector.tensor_copy(
                            out=ctx["xcat_T"][:, dt, 512 * qh:512 * (qh + 1)],
                            in_=tpb[:, 0:512])

                def emit_outproj_half(qh, tail=False):
                    for t in range(4):
                        ps = ppool.tile([128, 512], f32, tag="proj")
                        for c in range(2):
                            nc.tensor.matmul(
                                ps[:],
                                lhsT=ctx["wp_t"][:, c, 128 * t:128 * (t + 1)],
                                rhs=ctx["xcat_T"][:, c,
                                                  512 * qh:512 * (qh + 1)],
                                start=(c == 0), stop=(c == 1))
                        dst = ctx["out_sb"][:, t, 512 * qh:512 * (qh + 1)]
                        if tail and zb:
                            # ACT is idle after the last exp; Copy needs no
                            # activation table so there is no reload cost.
                            nc.scalar.activation(
                                dst, ps[:], mybir.ActivationFunctionType.Copy)
                        elif zb:
                            nc.vector.tensor_copy(out=dst, in_=ps[:])
                        else:
                            nc.vector.tensor_scalar(
                                out=dst, in0=ps[:],
                                scalar1=ctx["bp_t"][:, t:t + 1],
                                scalar2=None, op0=ADD)
                        nc.gpsimd.dma_start(
                            out=out_T[:].rearrange("(c p) n -> p c n", p=128)
                            [:, t, 512 * qh:512 * (qh + 1)],
                            in_=ctx["out_sb"][:, t, 512 * qh:512 * (qh + 1)])

                state = {"prev": None, "prev_ub": None}

                def emit_steps(extra=None, next_head=None):
                    for i, st in enumerate(steps):
                        npop = 2 if i < 5 else 1
                        prework = ctx["prework"]
                        for _ in range(npop):
                            if prework:
                                prework.pop(0)()
                        ub = emit_s(st)
                        prev, prev_ub = state["prev"], state["prev_ub"]
                        if prev is not None:
                            emit_pv(prev, prev_ub)
                            if prev[2] == 7:
                                emit_combine(prev[0], prev[1])
                                if prev[:2] == (1, 0):
                                    emit_transpose_half(0)
                                    emit_outproj_half(0)
                        state["prev"], state["prev_ub"] = st, ub
                        # previous rep's deferred tail fills early-C PE slack
                        if extra and i >= 1:
                            extra.pop(0)()
                        # next rep's input loads + Q/K projections go out
                        # after this rep's last bias DMA (no SP queue HOL)
                        if next_head is not None and i == 29:
                            next_head()

                def tail_parts():
                    prev, prev_ub = state["prev"], state["prev_ub"]

                    def part1():
                        emit_pv(prev, prev_ub)
                        emit_combine(prev[0], prev[1])

                    return [part1,
                            lambda: emit_transpose_half(1),
                            lambda: emit_outproj_half(1, tail=True)]

                ctx["head"] = head
                ctx["steps"] = emit_steps
                ctx["tail_parts"] = tail_parts
                return ctx

            # cross-rep software pipeline: the next rep's input DMAs and
            # Q/K projections are emitted before this rep's output tail, so
            # engines stay busy across the rep boundary.
            rctx = [make_rep() for _ in range(reps)]
            rctx[0]["head"]()
            deferred = []
            for r in range(reps):
                nh = rctx[r + 1]["head"] if r + 1 < reps else None
                rctx[r]["steps"](deferred, next_head=nh)
                deferred = rctx[r]["tail_parts"]()
            for f in deferred:
                f()
    nc.compile()
    return nc


def _get_kernel(reps=1, zb=True):
    key = f"k{reps}z{int(zb)}"
    if key not in _COMPILED:
        _COMPILED[key] = _build(reps, zb)
    return _COMPILED[key]


def _to_bf16(a):
    import jax.numpy as jnp
    return np.asarray(jnp.asarray(np.asarray(a), dtype=jnp.bfloat16))


def _zero_bias(bq, bk, bv, bp):
    return not (np.any(np.asarray(bq)) or np.any(np.asarray(bk))
                or np.any(np.asarray(bv)) or np.any(np.asarray(bp)))


def _prep_inputs(x_q, x_kv, coords_q, coords_k, alpha_map,
                 Wq, bq, Wk, bk, Wv, bv,
                 lambda_q1, lambda_k1, lambda_q2, lambda_k2,
                 rpe_table, Wp, bp, zb=None):
    if zb is None:
        zb = _zero_bias(bq, bk, bv, bp)
    x_q = np.asarray(x_q, dtype=np.float32)
    x_kv = np.asarray(x_kv, dtype=np.float32)
    coords_q = np.asarray(coords_q)
    coords_k = np.asarray(coords_k)
    alpha_map = np.asarray(alpha_map, dtype=np.float32)
    rpe = np.asarray(rpe_table, dtype=np.float32)
    Wq = np.asarray(Wq, dtype=np.float32)
    Wk = np.asarray(Wk, dtype=np.float32)
    Wv = np.asarray(Wv, dtype=np.float32)
    Wp = np.asarray(Wp, dtype=np.float32)
    bq = np.asarray(bq, dtype=np.float32)
    bk = np.asarray(bk, dtype=np.float32)
    bv = np.asarray(bv, dtype=np.float32)
    bp = np.asarray(bp, dtype=np.float32)

    lam1 = np.exp(np.sum(np.asarray(lambda_q1) * np.asarray(lambda_k1), axis=-1))
    lam2 = np.exp(np.sum(np.asarray(lambda_q2) * np.asarray(lambda_k2), axis=-1))
    lam = (lam1 - lam2 + LAMBDA_INIT).astype(np.float32)  # [4] per pair

    B = x_q.shape[0]
    # per-batch exp(bias) [q, k, H] and transposed bias, computed once
    expb_bT = []
    for b in range(B):
        rel = coords_q[b][:, None, :] - coords_k[b][None, :, :] + MAX_DIST
        rel = np.clip(rel, 0, 2 * MAX_DIST)
        idx = rel[..., 0] * (2 * MAX_DIST + 1) + rel[..., 1]  # [q, k]
        expb_bT.append(np.exp(rpe[idx]).transpose(2, 1, 0))  # [H, k, q]

    in_maps = []
    for c in range(N_CORES):
        b, hg = divmod(c, 2)
        hqk = [2 * hg, 2 * hg + 1, 2 * hg + 4, 2 * hg + 5]
        sl = lambda h: slice(64 * h, 64 * (h + 1))

        wq_l = np.concatenate([Wq.T[:, sl(h)] for h in hqk], 1) * SCALE
        wk_l = np.concatenate([Wk.T[:, sl(h)] for h in hqk], 1)
        # V col order per pair p: [V2 | V1] = heads [2hg+4+p, 2hg+p]
        hv = [2 * hg + 4, 2 * hg, 2 * hg + 5, 2 * hg + 1]
        wv_l = np.concatenate([Wv.T[:, sl(h)] for h in hv], 1)
        # xcat col order per pair p: [x1 | x2] = out dims [2hg+p, 2hg+4+p]
        hx = [2 * hg, 2 * hg + 4, 2 * hg + 1, 2 * hg + 5]
        wp_l = np.concatenate([Wp.T[sl(h), :] for h in hx], 0)

        bq_s = (np.concatenate([bq[sl(h)] for h in hqk]) * SCALE).reshape(2, 128).T
        bk_s = np.concatenate([bk[sl(h)] for h in hqk]).reshape(2, 128).T
        bqk_l = np.concatenate([bq_s, bk_s], 1)  # [128, 4]
        bv_s = np.concatenate([bv[sl(h)] for h in hv])
        bv_l = np.tile(bv_s[None, :], (128, 1))
        bp_l = bp.reshape(4, 128).T if hg == 0 else np.zeros((128, 4), np.float32)

        alpha_r = alpha_map[b, :, 0].reshape(8, 128).T  # [128, qt]
        alp_l = np.concatenate(
            [1.0 + alpha_r, alpha_r * lam[2 * hg], alpha_r * lam[2 * hg + 1]],
            1).reshape(128, 24, 1)

        # bias [pair, qhalf, ktile, k, 2*512]: head j of pair p, transposed
        eT = expb_bT[b]  # [H, k, q]
        bias_l = np.empty((2, 2, 8, 128, 2, 512), np.float32)
        for p in range(2):
            h1, h2 = 2 * hg + p, 2 * hg + 4 + p
            for qh in range(2):
                qs = slice(512 * qh, 512 * (qh + 1))
                bias_l[p, qh, :, :, 0, :] = eT[h1][:, qs].reshape(8, 128, 512)
                bias_l[p, qh, :, :, 1, :] = eT[h2][:, qs].reshape(8, 128, 512)
        bias_l = bias_l.reshape(2, 2, 8, 128, 1024)

        in_maps.append({
            "xq": _to_bf16(np.ascontiguousarray(x_q[b].T).reshape(4, 128, NQ)
                           .transpose(1, 0, 2)),
            "xkv": _to_bf16(np.ascontiguousarray(x_kv[b].T).reshape(4, 128, NKV)
                            .transpose(1, 0, 2)),
            "wq": _to_bf16(wq_l.reshape(4, 128, 256).transpose(1, 0, 2)),
            "wk": _to_bf16(wk_l.reshape(4, 128, 256).transpose(1, 0, 2)),
            "wv": _to_bf16(wv_l.reshape(4, 128, 256).transpose(1, 0, 2)),
            "wp": _to_bf16(wp_l.reshape(2, 128, DIM).transpose(1, 0, 2)),
            "alp": np.ascontiguousarray(alp_l),
            "biasT": _to_bf16(bias_l),
        })
        if not zb:
            in_maps[-1].update({
                "bqk": np.ascontiguousarray(bqk_l),
                "bv": np.ascontiguousarray(bv_l),
                "bp": np.ascontiguousarray(bp_l),
            })
    return in_maps


def kernel(x_q, x_kv, coords_q, coords_k, alpha_map,
           Wq, bq, Wk, bk, Wv, bv,
           lambda_q1, lambda_k1, lambda_q2, lambda_k2,
           rpe_table, Wp, bp):
    from concourse.bass_utils import run_bass_kernel_spmd

    zb = _zero_bias(bq, bk, bv, bp)
    nc = _get_kernel(zb=zb)
    in_maps = _prep_inputs(x_q, x_kv, coords_q, coords_k, alpha_map,
                           Wq, bq, Wk, bk, Wv, bv,
                           lambda_q1, lambda_k1, lambda_q2, lambda_k2,
                           rpe_table, Wp, bp, zb=zb)
    res = run_bass_kernel_spmd(nc, in_maps, list(range(N_CORES)))
    B = np.asarray(x_q).shape[0]
    out = np.zeros((B, NQ, DIM), dtype=np.float32)
    for b in range(B):
        out[b] = (res.results[2 * b]["out_T"].astype(np.float32) +
                  res.results[2 * b + 1]["out_T"].astype(np.float32)).T
    return out



# revision 2
# speedup vs baseline: 1.9409x; 1.9409x over previous
"""Differential cross-attention Trainium2 kernel (8 NeuronCores).

Sharding: 8 cores = (batch b = c//2) x (head-pair group hg = c%2). Each
core computes 2 differential pairs (heads 2hg+p, 2hg+4+p for p in 0,1)
over the FULL 1024 queries and 1024 keys of its batch, producing a
partial output projection (its 256 model dims through Wp); the two
partials per batch are summed on the host. This avoids recomputing K/V
projections on two cores (vs. a query-split).

Per-core layout (128-partition tiles):
  Q_T/K_T [do, n] fp32 via PE (4 accumulation passes over DIM), score
  matmuls in float32r (1 cycle/row at N=512). V [k, dv] bf16 arranged
  per pair as [V2 | V1 | 1] so the PV stage needs exactly 2 bf16
  matmuls per (pair, ktile, qtile): U1 @ [V1|1] -> [P1V1, S1] and
  U2 @ [V2|V1|1] -> [P2V2, P2V1, S2], accumulating over ktiles in PSUM.
  exp on ACT ([k,q] tiles, no max subtraction; |scores| <= ~2), RPE
  bias exp(rpe) multiplied post-exp on DVE/Pool in bf16. Combine folds
  softmax + differential into per-q scalars applied with broadcast APs;
  PE transposes feed the output projection.

Schedule (v2):
  - All input + bias DMAs issue up-front on the SP queue in priority
    order (wq, xq-h0, wk, xkv-h0, wv, then bias tiles interleaved with
    the remaining bulk); the DMA fabric drains one transfer at a time,
    so issue order is arrival order and the first score inputs land
    ~4us in. Output DMAs spread across the SP/Pool/ACT queues.
  - 7 junk warm-up matmuls (first rep) ramp the PE p-state to peak
    before the first real projection arrives.
  - Prework (remaining projections) is emitted AFTER each step's score
    matmuls so the first exp isn't stuck behind it in PE program order;
    deferred mid-kernel outproj chunks go out BEFORE the next step's
    scores (their deps are met, so they fill PE's wait on the score
    PSUM ring).
  - The transpose for the qh=0 output half is deferred one step so its
    wait on the DVE combine can't head-of-line block PE; outproj is
    spread one tile per step.
  - Pool runs a subset of bias multiplies (it cannot touch PSUM, so
    copies/combine stay on DVE/ACT), none in group 0 (head-of-line vs
    prework) or late in group 3 (tail critical path).
  - Tail output copies fan out ACT/DVE/ACT and the four output DMAs
    use distinct queues so the final drain is not serialized.
"""

import sys
sys.path.insert(0, "/opt/trn_rl_repo")
import numpy as np

DIM = 512
H = 8
HD = 64
NQ = 1024
NKV = 1024
MAX_DIST = 128
LAMBDA_INIT = 0.8
N_CORES = 8
SCALE = HD ** -0.5

_COMPILED = {}


def _build(reps=1, zb=True):
    import concourse.bacc as bacc
    import concourse.mybir as mybir
    from concourse.tile import TileContext
    from concourse.masks import make_identity

    f32 = mybir.dt.float32
    f32r = mybir.dt.float32r
    bf16 = mybir.dt.bfloat16
    ADD = mybir.AluOpType.add
    MUL = mybir.AluOpType.mult
    SUB = mybir.AluOpType.subtract
    EXP = mybir.ActivationFunctionType.Exp

    nc = bacc.Bacc("TRN2", target_bir_lowering=False, debug=False,
                   num_devices=N_CORES)

    xq = nc.dram_tensor("xq", [128, 4, NQ], bf16, kind="ExternalInput")
    xkv = nc.dram_tensor("xkv", [128, 4, NKV], bf16, kind="ExternalInput")
    wq = nc.dram_tensor("wq", [128, 4, 256], bf16, kind="ExternalInput")
    wk = nc.dram_tensor("wk", [128, 4, 256], bf16, kind="ExternalInput")
    wv = nc.dram_tensor("wv", [128, 4, 256], bf16, kind="ExternalInput")
    wp = nc.dram_tensor("wp", [128, 2, DIM], bf16, kind="ExternalInput")
    if not zb:
        bqk = nc.dram_tensor("bqk", [128, 4], f32, kind="ExternalInput")
        bv_d = nc.dram_tensor("bv", [128, 256], f32, kind="ExternalInput")
        bp_d = nc.dram_tensor("bp", [128, 4], f32, kind="ExternalInput")
    # packed per-q scalars: [:, 0:8] = 1+alpha, [:, 8:24] = alpha*lam[p]
    alp_d = nc.dram_tensor("alp", [128, 24, 1], f32, kind="ExternalInput")
    # [pair, qhalf, ktile, k, 2 heads * 512 q]
    biasT = nc.dram_tensor("biasT", [2, 2, 8, 128, NQ], bf16,
                           kind="ExternalInput")
    out_T = nc.dram_tensor("out_T", [DIM, NQ], bf16, kind="ExternalOutput")

    with TileContext(nc) as tc:
        with (
            tc.tile_pool(name="const", bufs=1) as cpool,
            tc.tile_pool(name="bias", bufs=6) as bpool,
            tc.tile_pool(name="u", bufs=3) as upool,
            tc.tile_pool(name="ub", bufs=3) as ubpool,
            tc.tile_pool(name="small", bufs=2) as rpool,
            tc.tile_pool(name="state", bufs=2) as stpool,
            tc.tile_pool(name="proj", bufs=2, space="PSUM") as ppool,
            tc.tile_pool(name="score", bufs=2, space="PSUM") as spool,
            tc.tile_pool(name="pvacc", bufs=1, space="PSUM") as pvpool,
        ):
            steps = [(p, qh, kt) for qh in range(2) for p in range(2)
                     for kt in range(8)]

            # identity is written once and shared by every rep (a per-rep
            # re-tile would be read-without-write for reps > 0)
            ident = cpool.tile([128, 128], bf16, tag="ident")
            make_identity(nc, ident[:])

            def make_rep(first=False):
                ctx = {}

                def head():
                    # tiles
                    wq_t = cpool.tile([128, 4, 256], bf16, tag="wq")
                    wk_t = cpool.tile([128, 4, 256], bf16, tag="wk")
                    wv_t = cpool.tile([128, 4, 256], bf16, tag="wv")
                    wp_t = cpool.tile([128, 2, DIM], bf16, tag="wp")
                    xq_t = stpool.tile([128, 4, NQ], bf16, tag="xq")
                    xkv_t = stpool.tile([128, 4, NKV], bf16, tag="xkv")
                    alp_t = cpool.tile([128, 24, 1], f32, tag="alp")
                    q_sb = stpool.tile([128, 2, NQ], f32r, tag="qsb")
                    k_sb = stpool.tile([128, 2, NKV], f32r, tag="ksb")
                    v_sb = stpool.tile([128, 8, 258], bf16, tag="vsb")
                    xcat = stpool.tile([128, 8, 256], bf16, tag="xcat")
                    xcat_T = stpool.tile([128, 2, NQ], bf16, tag="xcatT")
                    out_sb = stpool.tile([128, 4, NQ], bf16, tag="osb")
                    ctx.update(wq_t=wq_t, wk_t=wk_t, wv_t=wv_t, wp_t=wp_t,
                               xq_t=xq_t, xkv_t=xkv_t, alp_t=alp_t,
                               q_sb=q_sb, k_sb=k_sb, v_sb=v_sb, xcat=xcat,
                               xcat_T=xcat_T, out_sb=out_sb, ident=ident)
                    if not zb:
                        bqk_t = cpool.tile([128, 4], f32, tag="bqk")
                        bv_t = cpool.tile([128, 256], f32, tag="bv")
                        bp_t = cpool.tile([128, 4], f32, tag="bp")
                        ctx.update(bqk_t=bqk_t, bv_t=bv_t, bp_t=bp_t)
                        for t, s in ((bqk_t, bqk), (bv_t, bv_d), (bp_t, bp_d)):
                            nc.sync.dma_start(out=t[:], in_=s[:])
                    # all input + bias DMAs ride the SP queue, issued up
                    # front in priority order: the DMA fabric drains them
                    # serially so emission order here IS arrival order.
                    nc.sync.dma_start(out=wq_t[:], in_=wq[:])
                    nc.sync.dma_start(out=xq_t[:, :, 0:512], in_=xq[:, :, 0:512])
                    nc.sync.dma_start(out=wk_t[:], in_=wk[:])
                    nc.sync.dma_start(out=xkv_t[:, :, 0:512],
                                      in_=xkv[:, :, 0:512])
                    nc.sync.dma_start(out=wv_t[:], in_=wv[:])
                    emit_bias(0, 0)
                    nc.sync.dma_start(out=xkv_t[:, :, 512:1024],
                                      in_=xkv[:, :, 512:1024])
                    emit_bias(0, 1)
                    nc.sync.dma_start(out=xq_t[:, :, 512:1024],
                                      in_=xq[:, :, 512:1024])
                    nc.sync.dma_start(out=alp_t[:], in_=alp_d[:])
                    emit_bias(1, 0)
                    nc.sync.dma_start(out=ctx["wp_t"][:], in_=wp[:])
                    emit_bias(1, 1)
                    emit_bias(2, 0)
                    emit_bias(2, 1)
                    emit_bias(3, 0)
                    emit_bias(3, 1)
                    # warm-up matmuls (first rep only): ~4us of junk work
                    # ramps the PE p-state to peak before the first real
                    # projection; results land in the PV accumulator, which
                    # the first real PV epoch overwrites with start=True.
                    # Later reps keep PE busy across the boundary already.
                    if first:
                        warm = pvpool.tile([128, 4, 256], f32, tag="pv")
                        for w in range(7):
                            nc.tensor.matmul(
                                warm[:, 0:2, :], lhsT=ident[:],
                                rhs=ctx["v_sb"][:, 0:2, 0:256],
                                start=True, stop=True)
                        # BIR verifier requires every written location to
                        # have a reader; out_sb is fully overwritten by the
                        # real output copies before its DMA
                        nc.vector.tensor_copy(out=out_sb[:, 0, 0:1],
                                              in_=warm[:, 0, 0:1])
                    emit_q(0, 0)
                    emit_q(1, 0)
                    emit_k(0, 0)
                    emit_k(1, 0)
                    ctx["prework"] = (
                        [lambda: emit_k(0, 1), lambda: emit_k(1, 1)]
                        + [lambda kt=kt: emit_v(kt) for kt in range(3)]
                        + [lambda: emit_q(0, 1), lambda: emit_q(1, 1)]
                        + [lambda kt=kt: emit_v(kt) for kt in range(3, 8)])

                def emit_q(t, qh):
                    psp = ppool.tile([128, 512], f32, tag="proj")
                    ps = psp[:]
                    for c in range(4):
                        nc.tensor.matmul(
                            ps, lhsT=ctx["wq_t"][:, c, 128 * t:128 * (t + 1)],
                            rhs=ctx["xq_t"][:, c, 512 * qh:512 * (qh + 1)],
                            start=(c == 0), stop=(c == 3))
                    dst = ctx["q_sb"][:, t, 512 * qh:512 * (qh + 1)]
                    if zb:
                        nc.vector.tensor_copy(out=dst, in_=ps)
                    else:
                        nc.vector.tensor_scalar(
                            out=dst, in0=ps, scalar1=ctx["bqk_t"][:, t:t + 1],
                            scalar2=None, op0=ADD)

                def emit_k(t, qh):
                    psp = ppool.tile([128, 512], f32, tag="proj")
                    ps = psp[:]
                    for c in range(4):
                        nc.tensor.matmul(
                            ps, lhsT=ctx["wk_t"][:, c, 128 * t:128 * (t + 1)],
                            rhs=ctx["xkv_t"][:, c, 512 * qh:512 * (qh + 1)],
                            start=(c == 0), stop=(c == 3))
                    dst = ctx["k_sb"][:, t, 512 * qh:512 * (qh + 1)]
                    if zb:
                        nc.vector.tensor_copy(out=dst, in_=ps)
                    else:
                        nc.vector.tensor_scalar(
                            out=dst, in0=ps,
                            scalar1=ctx["bqk_t"][:, 2 + t:3 + t],
                            scalar2=None, op0=ADD)

                def emit_v(kt):
                    ps = ppool.tile([128, 512], f32, tag="proj")
                    for c in range(4):
                        nc.tensor.matmul(
                            ps[:, 0:256],
                            lhsT=ctx["xkv_t"][:, c, 128 * kt:128 * (kt + 1)],
                            rhs=ctx["wv_t"][:, c, :],
                            start=(c == 0), stop=(c == 3))
                    v_sb = ctx["v_sb"]
                    for p in range(2):
                        if zb:
                            nc.vector.tensor_copy(
                                out=v_sb[:, kt, 129 * p:129 * p + 128],
                                in_=ps[:, 128 * p:128 * (p + 1)])
                        else:
                            nc.vector.tensor_tensor(
                                out=v_sb[:, kt, 129 * p:129 * p + 128],
                                in0=ps[:, 128 * p:128 * (p + 1)],
                                in1=ctx["bv_t"][:, 128 * p:128 * (p + 1)],
                                op=ADD)
                        nc.gpsimd.memset(
                            v_sb[:, kt, 129 * p + 128:129 * p + 129], 1.0)

                pv_cur = {}
                bias_cur = {}
                # group order as visited by `steps`
                GROUPS = [(0, 0), (1, 0), (0, 1), (1, 1)]

                def emit_bias(g, half):
                    p, qh = GROUPS[g]
                    bt4 = bpool.tile([128, 4, 1024], bf16, tag="biasin")
                    nc.sync.dma_start(
                        out=bt4[:],
                        in_=biasT[p, qh, 4 * half:4 * half + 4, :, :]
                        .rearrange("t p n -> p t n"))
                    bias_cur[(p, qh, half)] = bt4

                ctx["emit_bias"] = emit_bias

                def emit_s(st):
                    p, qh, kt = st
                    g = GROUPS.index((p, qh))
                    ss = spool.tile([128, 1024], f32, tag="scores")
                    bt = bias_cur[(p, qh, kt // 4)][:, kt % 4, :]
                    for j in range(2):
                        nc.tensor.matmul(
                            ss[:, 512 * j:512 * (j + 1)],
                            lhsT=ctx["k_sb"][64 * p:64 * (p + 1), j,
                                             128 * kt:128 * (kt + 1)],
                            rhs=ctx["q_sb"][64 * p:64 * (p + 1), j,
                                            512 * qh:512 * (qh + 1)],
                            start=True, stop=True)
                    u = upool.tile([128, 1024], bf16, tag="u")
                    nc.scalar.activation(u[:], ss[:], EXP)
                    ub = ubpool.tile([128, 1024], bf16, tag="ub")
                    # offload some bias multiplies to the idle Pool engine —
                    # but none in group 0 (a slow Pool mult would head-of-line
                    # block the V copies Pool runs during prework) and none
                    # near the tail (critical path)
                    off = (kt == 2 and g > 0) or (kt == 6 and g in (1, 2))
                    eng = nc.gpsimd if off else nc.vector
                    eng.tensor_tensor(out=ub[:], in0=u[:], in1=bt, op=MUL)
                    return ub

                def emit_pv(st, ub):
                    p, qh, kt = st
                    if kt == 0:
                        pv_tile = pvpool.tile([128, 4, 256], f32, tag="pv")
                        pv_cur[(p, qh)] = pv_tile
                    pv = pv_cur[(p, qh)]
                    v_sb = ctx["v_sb"]
                    # start/stop are per-PSUM-bank epoch flags: one start
                    # (first matmul touching the bank zeroes it) and one stop
                    # (last of the epoch). Bank = qt//2 in this 2-bank tile.
                    for qt in range(4):
                        q0 = 128 * qt
                        nc.tensor.matmul(
                            pv[:, qt, 0:65], lhsT=ub[:, q0:q0 + 128],
                            rhs=v_sb[:, kt, 129 * p + 64:129 * p + 129],
                            start=(kt == 0 and qt % 2 == 0), stop=False)
                        nc.tensor.matmul(
                            pv[:, qt, 65:194],
                            lhsT=ub[:, 512 + q0:512 + q0 + 128],
                            rhs=v_sb[:, kt, 129 * p:129 * p + 129],
                            start=False, stop=(kt == 7 and qt % 2 == 1))

                def emit_combine(p, qh):
                    pv = pv_cur[(p, qh)]
                    al1_t = ctx["alp_t"][:, 0:8, :]
                    alam_t = ctx["alp_t"][:, 8:24, :]
                    xcat = ctx["xcat"]
                    rs1 = rpool.tile([128, 4, 1], f32, tag="rs1")
                    rs2 = rpool.tile([128, 4, 1], f32, tag="rs2")
                    g1 = rpool.tile([128, 4, 1], f32, tag="g1")
                    g2 = rpool.tile([128, 4, 1], f32, tag="g2")
                    nc.vector.reciprocal(rs1[:], pv[:, :, 64:65])
                    nc.vector.reciprocal(rs2[:], pv[:, :, 193:194])
                    nc.vector.tensor_tensor(
                        out=g1[:], in0=rs1[:],
                        in1=al1_t[:, 4 * qh:4 * qh + 4, :], op=MUL)
                    nc.vector.tensor_tensor(
                        out=g2[:], in0=rs2[:],
                        in1=alam_t[:, 8 * p + 4 * qh:8 * p + 4 * qh + 4, :],
                        op=MUL)
                    tmp1 = rpool.tile([128, 4, 64], f32, tag="tmp1")
                    tmp2 = rpool.tile([128, 4, 64], f32, tag="tmp2")
                    nc.vector.tensor_tensor(
                        out=tmp1[:], in0=pv[:, :, 0:64],
                        in1=g1[:].broadcast_to([128, 4, 64]), op=MUL)
                    nc.vector.tensor_tensor(
                        out=tmp2[:], in0=pv[:, :, 129:193],
                        in1=g2[:].broadcast_to([128, 4, 64]), op=MUL)
                    nc.vector.tensor_tensor(
                        out=xcat[:, 4 * qh:4 * qh + 4, 128 * p:128 * p + 64],
                        in0=tmp1[:], in1=tmp2[:], op=SUB)
                    nc.vector.tensor_tensor(
                        out=xcat[:, 4 * qh:4 * qh + 4,
                                 128 * p + 64:128 * (p + 1)],
                        in0=pv[:, :, 65:129],
                        in1=rs2[:].broadcast_to([128, 4, 64]), op=MUL)

                def emit_transpose_half(qh):
                    # 4 bf16 transposes packed per PSUM bank (per-bank epoch).
                    for dt in range(2):
                        tp = ppool.tile([128, 512], f32, tag="proj")
                        tpb = tp[:].bitcast(bf16)
                        for i in range(4):
                            qt = 4 * qh + i
                            nc.tensor.matmul(
                                tpb[:, 128 * i:128 * (i + 1)],
                                lhsT=ctx["xcat"][:, qt, 128 * dt:128 * (dt + 1)],
                                rhs=ctx["ident"][:], is_transpose=True,
                                start=(i == 0), stop=(i == 3))
                        nc.vector.tensor_copy(
                            out=ctx["xcat_T"][:, dt, 512 * qh:512 * (qh + 1)],
                            in_=tpb[:, 0:512])

                def emit_outproj_half(qh, tail=False, only_t=None):
                    # copies fan out across ACT/Pool/DVE and the DMAs across
                    # all four DGE queues so the drain isn't serialized on
                    # one engine at the end of the kernel
                    copy_eng = [nc.scalar, nc.vector, nc.vector, nc.scalar]
                    dma_eng = [nc.sync, nc.gpsimd, nc.sync, nc.scalar]
                    ts = range(4) if only_t is None else [only_t]
                    for t in ts:
                        ps = ppool.tile([128, 512], f32, tag="proj")
                        for c in range(2):
                            nc.tensor.matmul(
                                ps[:],
                                lhsT=ctx["wp_t"][:, c, 128 * t:128 * (t + 1)],
                                rhs=ctx["xcat_T"][:, c,
                                                  512 * qh:512 * (qh + 1)],
                                start=(c == 0), stop=(c == 1))
                        dst = ctx["out_sb"][:, t, 512 * qh:512 * (qh + 1)]
                        if tail and zb:
                            eng = copy_eng[t]
                            if eng is nc.scalar:
                                # Copy needs no activation table reload
                                nc.scalar.activation(
                                    dst, ps[:],
                                    mybir.ActivationFunctionType.Copy)
                            else:
                                eng.tensor_copy(out=dst, in_=ps[:])
                        elif zb:
                            nc.vector.tensor_copy(out=dst, in_=ps[:])
                        else:
                            nc.vector.tensor_scalar(
                                out=dst, in0=ps[:],
                                scalar1=ctx["bp_t"][:, t:t + 1],
                                scalar2=None, op0=ADD)
                        dma_eng[t].dma_start(
                            out=out_T[:].rearrange("(c p) n -> p c n", p=128)
                            [:, t, 512 * qh:512 * (qh + 1)],
                            in_=ctx["out_sb"][:, t, 512 * qh:512 * (qh + 1)])

                state = {"prev": None, "prev_ub": None}
                midq = []

                def emit_steps(extra=None, next_head=None):
                    for i, st in enumerate(steps):
                        # deferred work (outproj chunks, prev-rep tail) goes
                        # out BEFORE this step's score matmuls: its deps are
                        # already satisfied, so PE can chew on it while the
                        # scores wait for their PSUM buffer instead of the
                        # ready work being stuck behind them in-order
                        if midq:
                            midq.pop(0)()
                        if extra and i >= 1:
                            extra.pop(0)()
                        ub = emit_s(st)
                        npop = 2 if i < 5 else 1
                        prework = ctx["prework"]
                        for _ in range(npop):
                            if prework:
                                prework.pop(0)()
                        prev, prev_ub = state["prev"], state["prev_ub"]
                        if prev is not None:
                            emit_pv(prev, prev_ub)
                            if prev[2] == 7:
                                emit_combine(prev[0], prev[1])
                                if prev[:2] == (1, 0):
                                    # defer so the transpose (which waits on
                                    # the DVE combine) can't head-of-line
                                    # block the next steps' score matmuls on
                                    # PE; spread outproj over later steps
                                    midq.append(
                                        lambda: emit_transpose_half(0))
                                    for t in range(4):
                                        midq.append(
                                            lambda t=t:
                                            emit_outproj_half(0, only_t=t))
                        state["prev"], state["prev_ub"] = st, ub
                        # next rep's input loads + Q/K projections go out
                        # after this rep's last bias DMA (no SP queue HOL)
                        if next_head is not None and i == 29:
                            next_head()

                def tail_parts():
                    prev, prev_ub = state["prev"], state["prev_ub"]

                    def part1():
                        emit_pv(prev, prev_ub)
                        emit_combine(prev[0], prev[1])

                    return [part1,
                            lambda: emit_transpose_half(1),
                            lambda: emit_outproj_half(1, tail=True)]

                ctx["head"] = head
                ctx["steps"] = emit_steps
                ctx["tail_parts"] = tail_parts
                return ctx

            # cross-rep software pipeline: the next rep's input DMAs and
            # Q/K projections are emitted before this rep's output tail, so
            # engines stay busy across the rep boundary.
            rctx = [make_rep(first=(r == 0)) for r in range(reps)]
            rctx[0]["head"]()
            deferred = []
            for r in range(reps):
                nh = rctx[r + 1]["head"] if r + 1 < reps else None
                rctx[r]["steps"](deferred, next_head=nh)
                deferred = rctx[r]["tail_parts"]()
            for f in deferred:
                f()
    nc.compile()
    return nc


def _get_kernel(reps=1, zb=True):
    key = f"k{reps}z{int(zb)}"
    if key not in _COMPILED:
        _COMPILED[key] = _build(reps, zb)
    return _COMPILED[key]


def _to_bf16(a):
    import jax.numpy as jnp
    return np.asarray(jnp.asarray(np.asarray(a), dtype=jnp.bfloat16))


def _zero_bias(bq, bk, bv, bp):
    return not (np.any(np.asarray(bq)) or np.any(np.asarray(bk))
                or np.any(np.asarray(bv)) or np.any(np.asarray(bp)))


def _prep_inputs(x_q, x_kv, coords_q, coords_k, alpha_map,
                 Wq, bq, Wk, bk, Wv, bv,
                 lambda_q1, lambda_k1, lambda_q2, lambda_k2,
                 rpe_table, Wp, bp, zb=None):
    if zb is None:
        zb = _zero_bias(bq, bk, bv, bp)
    x_q = np.asarray(x_q, dtype=np.float32)
    x_kv = np.asarray(x_kv, dtype=np.float32)
    coords_q = np.asarray(coords_q)
    coords_k = np.asarray(coords_k)
    alpha_map = np.asarray(alpha_map, dtype=np.float32)
    rpe = np.asarray(rpe_table, dtype=np.float32)
    Wq = np.asarray(Wq, dtype=np.float32)
    Wk = np.asarray(Wk, dtype=np.float32)
    Wv = np.asarray(Wv, dtype=np.float32)
    Wp = np.asarray(Wp, dtype=np.float32)
    bq = np.asarray(bq, dtype=np.float32)
    bk = np.asarray(bk, dtype=np.float32)
    bv = np.asarray(bv, dtype=np.float32)
    bp = np.asarray(bp, dtype=np.float32)

    lam1 = np.exp(np.sum(np.asarray(lambda_q1) * np.asarray(lambda_k1), axis=-1))
    lam2 = np.exp(np.sum(np.asarray(lambda_q2) * np.asarray(lambda_k2), axis=-1))
    lam = (lam1 - lam2 + LAMBDA_INIT).astype(np.float32)  # [4] per pair

    B = x_q.shape[0]
    # per-batch exp(bias) [q, k, H] and transposed bias, computed once
    expb_bT = []
    for b in range(B):
        rel = coords_q[b][:, None, :] - coords_k[b][None, :, :] + MAX_DIST
        rel = np.clip(rel, 0, 2 * MAX_DIST)
        idx = rel[..., 0] * (2 * MAX_DIST + 1) + rel[..., 1]  # [q, k]
        expb_bT.append(np.exp(rpe[idx]).transpose(2, 1, 0))  # [H, k, q]

    in_maps = []
    for c in range(N_CORES):
        b, hg = divmod(c, 2)
        hqk = [2 * hg, 2 * hg + 1, 2 * hg + 4, 2 * hg + 5]
        sl = lambda h: slice(64 * h, 64 * (h + 1))

        wq_l = np.concatenate([Wq.T[:, sl(h)] for h in hqk], 1) * SCALE
        wk_l = np.concatenate([Wk.T[:, sl(h)] for h in hqk], 1)
        # V col order per pair p: [V2 | V1] = heads [2hg+4+p, 2hg+p]
        hv = [2 * hg + 4, 2 * hg, 2 * hg + 5, 2 * hg + 1]
        wv_l = np.concatenate([Wv.T[:, sl(h)] for h in hv], 1)
        # xcat col order per pair p: [x1 | x2] = out dims [2hg+p, 2hg+4+p]
        hx = [2 * hg, 2 * hg + 4, 2 * hg + 1, 2 * hg + 5]
        wp_l = np.concatenate([Wp.T[sl(h), :] for h in hx], 0)

        bq_s = (np.concatenate([bq[sl(h)] for h in hqk]) * SCALE).reshape(2, 128).T
        bk_s = np.concatenate([bk[sl(h)] for h in hqk]).reshape(2, 128).T
        bqk_l = np.concatenate([bq_s, bk_s], 1)  # [128, 4]
        bv_s = np.concatenate([bv[sl(h)] for h in hv])
        bv_l = np.tile(bv_s[None, :], (128, 1))
        bp_l = bp.reshape(4, 128).T if hg == 0 else np.zeros((128, 4), np.float32)

        alpha_r = alpha_map[b, :, 0].reshape(8, 128).T  # [128, qt]
        alp_l = np.concatenate(
            [1.0 + alpha_r, alpha_r * lam[2 * hg], alpha_r * lam[2 * hg + 1]],
            1).reshape(128, 24, 1)

        # bias [pair, qhalf, ktile, k, 2*512]: head j of pair p, transposed
        eT = expb_bT[b]  # [H, k, q]
        bias_l = np.empty((2, 2, 8, 128, 2, 512), np.float32)
        for p in range(2):
            h1, h2 = 2 * hg + p, 2 * hg + 4 + p
            for qh in range(2):
                qs = slice(512 * qh, 512 * (qh + 1))
                bias_l[p, qh, :, :, 0, :] = eT[h1][:, qs].reshape(8, 128, 512)
                bias_l[p, qh, :, :, 1, :] = eT[h2][:, qs].reshape(8, 128, 512)
        bias_l = bias_l.reshape(2, 2, 8, 128, 1024)

        in_maps.append({
            "xq": _to_bf16(np.ascontiguousarray(x_q[b].T).reshape(4, 128, NQ)
                           .transpose(1, 0, 2)),
            "xkv": _to_bf16(np.ascontiguousarray(x_kv[b].T).reshape(4, 128, NKV)
                            .transpose(1, 0, 2)),
            "wq": _to_bf16(wq_l.reshape(4, 128, 256).transpose(1, 0, 2)),
            "wk": _to_bf16(wk_l.reshape(4, 128, 256).transpose(1, 0, 2)),
            "wv": _to_bf16(wv_l.reshape(4, 128, 256).transpose(1, 0, 2)),
            "wp": _to_bf16(wp_l.reshape(2, 128, DIM).transpose(1, 0, 2)),
            "alp": np.ascontiguousarray(alp_l),
            "biasT": _to_bf16(bias_l),
        })
        if not zb:
            in_maps[-1].update({
                "bqk": np.ascontiguousarray(bqk_l),
                "bv": np.ascontiguousarray(bv_l),
                "bp": np.ascontiguousarray(bp_l),
            })
    return in_maps


def kernel(x_q, x_kv, coords_q, coords_k, alpha_map,
           Wq, bq, Wk, bk, Wv, bv,
           lambda_q1, lambda_k1, lambda_q2, lambda_k2,
           rpe_table, Wp, bp):
    from concourse.bass_utils import run_bass_kernel_spmd

    zb = _zero_bias(bq, bk, bv, bp)
    nc = _get_kernel(zb=zb)
    in_maps = _prep_inputs(x_q, x_kv, coords_q, coords_k, alpha_map,
                           Wq, bq, Wk, bk, Wv, bv,
                           lambda_q1, lambda_k1, lambda_q2, lambda_k2,
                           rpe_table, Wp, bp, zb=zb)
    res = run_bass_kernel_spmd(nc, in_maps, list(range(N_CORES)))
    B = np.asarray(x_q).shape[0]
    out = np.zeros((B, NQ, DIM), dtype=np.float32)
    for b in range(B):
        out[b] = (res.results[2 * b]["out_T"].astype(np.float32) +
                  res.results[2 * b + 1]["out_T"].astype(np.float32)).T
    return out



# revision 3
# speedup vs baseline: 2.4026x; 1.2379x over previous
"""Differential cross-attention Trainium2 kernel (8 NeuronCores).

Sharding: 8 cores = (batch b = c//2) x (head-pair group hg = c%2). Each
core computes 2 differential pairs (heads 2hg+p, 2hg+4+p for p in 0,1)
over the FULL 1024 queries and 1024 keys of its batch, producing a
partial output projection (its 256 model dims through Wp); the two
partials per batch are summed on the host. This avoids recomputing K/V
projections on two cores (vs. a query-split).

Per-core layout (128-partition tiles):
  Q_T/K_T [do, n] fp32 via PE (4 accumulation passes over DIM), score
  matmuls in float32r (1 cycle/row at N=512). V [k, dv] bf16 arranged
  per pair as [V2 | V1 | 1] so the PV stage needs exactly 2 bf16
  matmuls per (pair, ktile, qtile): U1 @ [V1|1] -> [P1V1, S1] and
  U2 @ [V2|V1|1] -> [P2V2, P2V1, S2], accumulating over ktiles in PSUM.
  exp on ACT ([k,q] tiles, no max subtraction; |scores| <= ~2), RPE
  bias exp(rpe) multiplied post-exp on DVE/Pool in bf16. Combine folds
  softmax + differential into per-q scalars applied with broadcast APs;
  PE transposes feed the output projection.

Schedule (v2):
  - All input + bias DMAs issue up-front on the SP queue in priority
    order (wq, xq-h0, wk, xkv-h0, wv, then bias tiles interleaved with
    the remaining bulk); the DMA fabric drains one transfer at a time,
    so issue order is arrival order and the first score inputs land
    ~4us in. Output DMAs spread across the SP/Pool/ACT queues.
  - 5 junk warm-up matmuls (first rep) ramp the PE p-state to peak
    before the first real projection arrives.
  - Prework (remaining projections) is emitted AFTER each step's score
    matmuls so the first exp isn't stuck behind it in PE program order;
    deferred mid-kernel outproj chunks go out BEFORE the next step's
    scores (their deps are met, so they fill PE's wait on the score
    PSUM ring).
  - The transpose for the qh=0 output half is deferred one step so its
    wait on the DVE combine can't head-of-line block PE; outproj is
    spread one tile per step.
  - Pool runs a subset of bias multiplies (it cannot touch PSUM, so
    copies/combine stay on DVE/ACT), none in group 0 (head-of-line vs
    prework) or late in group 3 (tail critical path).
  - Tail output copies fan out ACT/DVE/ACT and the four output DMAs
    use distinct queues so the final drain is not serialized.
"""
import sys
sys.path.insert(0, "/opt/trn_rl_repo")
import numpy as np

DIM = 512
H = 8
HD = 64
NQ = 1024
NKV = 1024
MAX_DIST = 128
LAMBDA_INIT = 0.8
N_CORES = 8
SCALE = HD ** -0.5

_COMPILED = {}


def _build(reps=1, zb=True):
    import concourse.bacc as bacc
    import concourse.mybir as mybir
    from concourse.tile import TileContext
    from concourse.masks import make_identity

    f32 = mybir.dt.float32
    f32r = mybir.dt.float32r
    bf16 = mybir.dt.bfloat16
    ADD = mybir.AluOpType.add
    MUL = mybir.AluOpType.mult
    SUB = mybir.AluOpType.subtract
    EXP = mybir.ActivationFunctionType.Exp

    nc = bacc.Bacc("TRN2", target_bir_lowering=False, debug=False,
                   num_devices=N_CORES)

    xq = nc.dram_tensor("xq", [128, 4, NQ], bf16, kind="ExternalInput")
    xkv = nc.dram_tensor("xkv", [128, 4, NKV], bf16, kind="ExternalInput")
    wq = nc.dram_tensor("wq", [128, 4, 256], bf16, kind="ExternalInput")
    wk = nc.dram_tensor("wk", [128, 4, 256], bf16, kind="ExternalInput")
    wv = nc.dram_tensor("wv", [128, 4, 256], bf16, kind="ExternalInput")
    wp = nc.dram_tensor("wp", [128, 2, DIM], bf16, kind="ExternalInput")
    if not zb:
        bqk = nc.dram_tensor("bqk", [128, 4], f32, kind="ExternalInput")
        bv_d = nc.dram_tensor("bv", [128, 256], f32, kind="ExternalInput")
        bp_d = nc.dram_tensor("bp", [128, 4], f32, kind="ExternalInput")
    # packed per-q scalars: [:, 0:8] = 1+alpha, [:, 8:24] = alpha*lam[p]
    alp_d = nc.dram_tensor("alp", [128, 24, 1], f32, kind="ExternalInput")
    # [pair, qhalf, ktile, k, 2 heads * 512 q]
    biasT = nc.dram_tensor("biasT", [2, 2, 8, 128, NQ], bf16,
                           kind="ExternalInput")
    out_T = nc.dram_tensor("out_T", [DIM, NQ], bf16, kind="ExternalOutput")

    with TileContext(nc) as tc:
        with (
            tc.tile_pool(name="const", bufs=1) as cpool,
            tc.tile_pool(name="bias", bufs=6) as bpool,
            tc.tile_pool(name="u", bufs=4) as upool,
            tc.tile_pool(name="ub", bufs=3) as ubpool,
            tc.tile_pool(name="small", bufs=2) as rpool,
            tc.tile_pool(name="state", bufs=2) as stpool,
            tc.tile_pool(name="proj", bufs=2, space="PSUM") as ppool,
            tc.tile_pool(name="score", bufs=2, space="PSUM") as spool,
            tc.tile_pool(name="pvacc", bufs=1, space="PSUM") as pvpool,
        ):
            steps = [(p, qh, kt) for qh in range(2) for p in range(2)
                     for kt in range(8)]

            # identity is written once and shared by every rep (a per-rep
            # re-tile would be read-without-write for reps > 0)
            ident = cpool.tile([128, 128], bf16, tag="ident")
            make_identity(nc, ident[:])

            def make_rep(first=False):
                ctx = {}

                def head():
                    # tiles
                    wq_t = cpool.tile([128, 4, 256], bf16, tag="wq")
                    wk_t = cpool.tile([128, 4, 256], bf16, tag="wk")
                    wv_t = cpool.tile([128, 4, 256], bf16, tag="wv")
                    wp_t = cpool.tile([128, 2, DIM], bf16, tag="wp")
                    xq_t = stpool.tile([128, 4, NQ], bf16, tag="xq")
                    xkv_t = stpool.tile([128, 4, NKV], bf16, tag="xkv")
                    alp_t = cpool.tile([128, 24, 1], f32, tag="alp")
                    q_sb = stpool.tile([128, 2, NQ], f32r, tag="qsb")
                    k_sb = stpool.tile([128, 2, NKV], f32r, tag="ksb")
                    v_sb = stpool.tile([128, 8, 258], bf16, tag="vsb")
                    xcat = stpool.tile([128, 8, 256], bf16, tag="xcat")
                    xcat_T = stpool.tile([128, 2, NQ], bf16, tag="xcatT")
                    out_sb = stpool.tile([128, 4, NQ], bf16, tag="osb")
                    ctx.update(wq_t=wq_t, wk_t=wk_t, wv_t=wv_t, wp_t=wp_t,
                               xq_t=xq_t, xkv_t=xkv_t, alp_t=alp_t,
                               q_sb=q_sb, k_sb=k_sb, v_sb=v_sb, xcat=xcat,
                               xcat_T=xcat_T, out_sb=out_sb, ident=ident)
                    if not zb:
                        bqk_t = cpool.tile([128, 4], f32, tag="bqk")
                        bv_t = cpool.tile([128, 256], f32, tag="bv")
                        bp_t = cpool.tile([128, 4], f32, tag="bp")
                        ctx.update(bqk_t=bqk_t, bv_t=bv_t, bp_t=bp_t)
                        for t, s in ((bqk_t, bqk), (bv_t, bv_d), (bp_t, bp_d)):
                            nc.sync.dma_start(out=t[:], in_=s[:])
                    # all input + bias DMAs ride the SP queue, issued up
                    # front in priority order: the DMA fabric drains them
                    # serially so emission order here IS arrival order.
                    nc.sync.dma_start(out=wq_t[:], in_=wq[:])
                    nc.sync.dma_start(out=xq_t[:, :, 0:512], in_=xq[:, :, 0:512])
                    nc.sync.dma_start(out=wk_t[:], in_=wk[:])
                    nc.sync.dma_start(out=xkv_t[:, :, 0:512],
                                      in_=xkv[:, :, 0:512])
                    nc.sync.dma_start(out=wv_t[:], in_=wv[:])
                    emit_bias(0, 0)
                    nc.sync.dma_start(out=xkv_t[:, :, 512:1024],
                                      in_=xkv[:, :, 512:1024])
                    emit_bias(0, 1)
                    nc.sync.dma_start(out=xq_t[:, :, 512:1024],
                                      in_=xq[:, :, 512:1024])
                    nc.sync.dma_start(out=alp_t[:], in_=alp_d[:])
                    emit_bias(1, 0)
                    nc.sync.dma_start(out=ctx["wp_t"][:], in_=wp[:])
                    emit_bias(1, 1)
                    emit_bias(2, 0)
                    emit_bias(2, 1)
                    emit_bias(3, 0)
                    emit_bias(3, 1)
                    # warm-up matmuls (first rep only): ~4us of junk work
                    # ramps the PE p-state to peak before the first real
                    # projection; results land in the PV accumulator, which
                    # the first real PV epoch overwrites with start=True.
                    # Later reps keep PE busy across the boundary already.
                    if first:
                        warm = pvpool.tile([128, 4, 256], f32, tag="pv")
                        for w in range(5):
                            nc.tensor.matmul(
                                warm[:, 0:2, :], lhsT=ident[:],
                                rhs=ctx["v_sb"][:, 0:2, 0:256],
                                start=True, stop=True)
                        # BIR verifier requires every written location to
                        # have a reader; out_sb is fully overwritten by the
                        # real output copies before its DMA
                        nc.vector.tensor_copy(out=out_sb[:, 0, 0:1],
                                              in_=warm[:, 0, 0:1])
                    emit_q(0, 0)
                    emit_q(1, 0)
                    emit_k(0, 0)
                    emit_k(1, 0)
                    ctx["prework"] = (
                        [lambda: emit_k(0, 1), lambda: emit_k(1, 1)]
                        + [lambda kt=kt: emit_v(kt) for kt in range(3)]
                        + [lambda: emit_q(0, 1), lambda: emit_q(1, 1)]
                        + [lambda kt=kt: emit_v(kt) for kt in range(3, 8)])

                def emit_q(t, qh):
                    psp = ppool.tile([128, 512], f32, tag="proj")
                    ps = psp[:]
                    for c in range(4):
                        nc.tensor.matmul(
                            ps, lhsT=ctx["wq_t"][:, c, 128 * t:128 * (t + 1)],
                            rhs=ctx["xq_t"][:, c, 512 * qh:512 * (qh + 1)],
                            start=(c == 0), stop=(c == 3))
                    dst = ctx["q_sb"][:, t, 512 * qh:512 * (qh + 1)]
                    if zb:
                        nc.vector.tensor_copy(out=dst, in_=ps)
                    else:
                        nc.vector.tensor_scalar(
                            out=dst, in0=ps, scalar1=ctx["bqk_t"][:, t:t + 1],
                            scalar2=None, op0=ADD)

                def emit_k(t, qh):
                    psp = ppool.tile([128, 512], f32, tag="proj")
                    ps = psp[:]
                    for c in range(4):
                        nc.tensor.matmul(
                            ps, lhsT=ctx["wk_t"][:, c, 128 * t:128 * (t + 1)],
                            rhs=ctx["xkv_t"][:, c, 512 * qh:512 * (qh + 1)],
                            start=(c == 0), stop=(c == 3))
                    dst = ctx["k_sb"][:, t, 512 * qh:512 * (qh + 1)]
                    if zb:
                        nc.vector.tensor_copy(out=dst, in_=ps)
                    else:
                        nc.vector.tensor_scalar(
                            out=dst, in0=ps,
                            scalar1=ctx["bqk_t"][:, 2 + t:3 + t],
                            scalar2=None, op0=ADD)

                def emit_v(kt):
                    ps = ppool.tile([128, 512], f32, tag="proj")
                    for c in range(4):
                        nc.tensor.matmul(
                            ps[:, 0:256],
                            lhsT=ctx["xkv_t"][:, c, 128 * kt:128 * (kt + 1)],
                            rhs=ctx["wv_t"][:, c, :],
                            start=(c == 0), stop=(c == 3))
                    v_sb = ctx["v_sb"]
                    for p in range(2):
                        if zb:
                            nc.vector.tensor_copy(
                                out=v_sb[:, kt, 129 * p:129 * p + 128],
                                in_=ps[:, 128 * p:128 * (p + 1)])
                        else:
                            nc.vector.tensor_tensor(
                                out=v_sb[:, kt, 129 * p:129 * p + 128],
                                in0=ps[:, 128 * p:128 * (p + 1)],
                                in1=ctx["bv_t"][:, 128 * p:128 * (p + 1)],
                                op=ADD)
                        nc.gpsimd.memset(
                            v_sb[:, kt, 129 * p + 128:129 * p + 129], 1.0)

                pv_cur = {}
                bias_cur = {}
                # group order as visited by `steps`
                GROUPS = [(0, 0), (1, 0), (0, 1), (1, 1)]

                def emit_bias(g, half):
                    p, qh = GROUPS[g]
                    bt4 = bpool.tile([128, 4, 1024], bf16, tag="biasin")
                    nc.sync.dma_start(
                        out=bt4[:],
                        in_=biasT[p, qh, 4 * half:4 * half + 4, :, :]
                        .rearrange("t p n -> p t n"))
                    bias_cur[(p, qh, half)] = bt4

                ctx["emit_bias"] = emit_bias

                def emit_s(st):
                    p, qh, kt = st
                    g = GROUPS.index((p, qh))
                    ss = spool.tile([128, 1024], f32, tag="scores")
                    bt = bias_cur[(p, qh, kt // 4)][:, kt % 4, :]
                    for j in range(2):
                        nc.tensor.matmul(
                            ss[:, 512 * j:512 * (j + 1)],
                            lhsT=ctx["k_sb"][64 * p:64 * (p + 1), j,
                                             128 * kt:128 * (kt + 1)],
                            rhs=ctx["q_sb"][64 * p:64 * (p + 1), j,
                                            512 * qh:512 * (qh + 1)],
                            start=True, stop=True)
                    u = upool.tile([128, 1024], bf16, tag="u")
                    nc.scalar.activation(u[:], ss[:], EXP)
                    ub = ubpool.tile([128, 1024], bf16, tag="ub")
                    # offload some bias multiplies to the idle Pool engine —
                    # but none in group 0 (a slow Pool mult would head-of-line
                    # block the V copies Pool runs during prework) and none
                    # near the tail (critical path)
                    off = (kt == 2 and g > 0) or (kt == 6 and g in (1, 2))
                    eng = nc.gpsimd if off else nc.vector
                    eng.tensor_tensor(out=ub[:], in0=u[:], in1=bt, op=MUL)
                    return ub

                def emit_pv(st, ub):
                    p, qh, kt = st
                    if kt == 0:
                        pv_tile = pvpool.tile([128, 4, 256], f32, tag="pv")
                        pv_cur[(p, qh)] = pv_tile
                    pv = pv_cur[(p, qh)]
                    v_sb = ctx["v_sb"]
                    # start/stop are per-PSUM-bank epoch flags: one start
                    # (first matmul touching the bank zeroes it) and one stop
                    # (last of the epoch). Bank = qt//2 in this 2-bank tile.
                    for qt in range(4):
                        q0 = 128 * qt
                        nc.tensor.matmul(
                            pv[:, qt, 0:65], lhsT=ub[:, q0:q0 + 128],
                            rhs=v_sb[:, kt, 129 * p + 64:129 * p + 129],
                            start=(kt == 0 and qt % 2 == 0), stop=False)
                        nc.tensor.matmul(
                            pv[:, qt, 65:194],
                            lhsT=ub[:, 512 + q0:512 + q0 + 128],
                            rhs=v_sb[:, kt, 129 * p:129 * p + 129],
                            start=False, stop=(kt == 7 and qt % 2 == 1))

                def emit_combine(p, qh):
                    pv = pv_cur[(p, qh)]
                    al1_t = ctx["alp_t"][:, 0:8, :]
                    alam_t = ctx["alp_t"][:, 8:24, :]
                    xcat = ctx["xcat"]
                    rs1 = rpool.tile([128, 4, 1], f32, tag="rs1")
                    rs2 = rpool.tile([128, 4, 1], f32, tag="rs2")
                    g1 = rpool.tile([128, 4, 1], f32, tag="g1")
                    g2 = rpool.tile([128, 4, 1], f32, tag="g2")
                    nc.vector.reciprocal(rs1[:], pv[:, :, 64:65])
                    nc.vector.reciprocal(rs2[:], pv[:, :, 193:194])
                    nc.vector.tensor_tensor(
                        out=g1[:], in0=rs1[:],
                        in1=al1_t[:, 4 * qh:4 * qh + 4, :], op=MUL)
                    nc.vector.tensor_tensor(
                        out=g2[:], in0=rs2[:],
                        in1=alam_t[:, 8 * p + 4 * qh:8 * p + 4 * qh + 4, :],
                        op=MUL)
                    tmp1 = rpool.tile([128, 4, 64], f32, tag="tmp1")
                    tmp2 = rpool.tile([128, 4, 64], f32, tag="tmp2")
                    nc.vector.tensor_tensor(
                        out=tmp1[:], in0=pv[:, :, 0:64],
                        in1=g1[:].broadcast_to([128, 4, 64]), op=MUL)
                    nc.vector.tensor_tensor(
                        out=tmp2[:], in0=pv[:, :, 129:193],
                        in1=g2[:].broadcast_to([128, 4, 64]), op=MUL)
                    nc.vector.tensor_tensor(
                        out=xcat[:, 4 * qh:4 * qh + 4, 128 * p:128 * p + 64],
                        in0=tmp1[:], in1=tmp2[:], op=SUB)
                    nc.vector.tensor_tensor(
                        out=xcat[:, 4 * qh:4 * qh + 4,
                                 128 * p + 64:128 * (p + 1)],
                        in0=pv[:, :, 65:129],
                        in1=rs2[:].broadcast_to([128, 4, 64]), op=MUL)

                def emit_transpose_half(qh):
                    # 4 bf16 transposes packed per PSUM bank (per-bank epoch).
                    for dt in range(2):
                        tp = ppool.tile([128, 512], f32, tag="proj")
                        tpb = tp[:].bitcast(bf16)
                        for i in range(4):
                            qt = 4 * qh + i
                            nc.tensor.matmul(
                                tpb[:, 128 * i:128 * (i + 1)],
                                lhsT=ctx["xcat"][:, qt, 128 * dt:128 * (dt + 1)],
                                rhs=ctx["ident"][:], is_transpose=True,
                                start=(i == 0), stop=(i == 3))
                        nc.vector.tensor_copy(
                            out=ctx["xcat_T"][:, dt, 512 * qh:512 * (qh + 1)],
                            in_=tpb[:, 0:512])

                def emit_outproj_half(qh, tail=False, only_t=None):
                    # copies fan out across ACT/Pool/DVE and the DMAs across
                    # all four DGE queues so the drain isn't serialized on
                    # one engine at the end of the kernel
                    copy_eng = [nc.scalar, nc.vector, nc.vector, nc.scalar]
                    dma_eng = [nc.sync, nc.gpsimd, nc.sync, nc.scalar]
                    ts = range(4) if only_t is None else [only_t]
                    for t in ts:
                        ps = ppool.tile([128, 512], f32, tag="proj")
                        for c in range(2):
                            nc.tensor.matmul(
                                ps[:],
                                lhsT=ctx["wp_t"][:, c, 128 * t:128 * (t + 1)],
                                rhs=ctx["xcat_T"][:, c,
                                                  512 * qh:512 * (qh + 1)],
                                start=(c == 0), stop=(c == 1))
                        dst = ctx["out_sb"][:, t, 512 * qh:512 * (qh + 1)]
                        if tail and zb:
                            eng = copy_eng[t]
                            if eng is nc.scalar:
                                # Copy needs no activation table reload
                                nc.scalar.activation(
                                    dst, ps[:],
                                    mybir.ActivationFunctionType.Copy)
                            else:
                                eng.tensor_copy(out=dst, in_=ps[:])
                        elif zb:
                            nc.vector.tensor_copy(out=dst, in_=ps[:])
                        else:
                            nc.vector.tensor_scalar(
                                out=dst, in0=ps[:],
                                scalar1=ctx["bp_t"][:, t:t + 1],
                                scalar2=None, op0=ADD)
                        dma_eng[t].dma_start(
                            out=out_T[:].rearrange("(c p) n -> p c n", p=128)
                            [:, t, 512 * qh:512 * (qh + 1)],
                            in_=ctx["out_sb"][:, t, 512 * qh:512 * (qh + 1)])

                state = {"prev": None, "prev_ub": None}
                midq = []

                def emit_steps(extra=None, next_head=None):
                    for i, st in enumerate(steps):
                        # deferred work (outproj chunks, prev-rep tail) goes
                        # out BEFORE this step's score matmuls: its deps are
                        # already satisfied, so PE can chew on it while the
                        # scores wait for their PSUM buffer instead of the
                        # ready work being stuck behind them in-order
                        if midq:
                            midq.pop(0)()
                        if extra and i >= 1:
                            extra.pop(0)()
                        ub = emit_s(st)
                        npop = 2 if i < 5 else 1
                        prework = ctx["prework"]
                        for _ in range(npop):
                            if prework:
                                prework.pop(0)()
                        prev, prev_ub = state["prev"], state["prev_ub"]
                        if prev is not None:
                            emit_pv(prev, prev_ub)
                            if prev[2] == 7:
                                emit_combine(prev[0], prev[1])
                                if prev[:2] == (1, 0):
                                    # defer so the transpose (which waits on
                                    # the DVE combine) can't head-of-line
                                    # block the next steps' score matmuls on
                                    # PE; spread outproj over later steps
                                    midq.append(
                                        lambda: emit_transpose_half(0))
                                    for t in range(4):
                                        midq.append(
                                            lambda t=t:
                                            emit_outproj_half(0, only_t=t))
                        state["prev"], state["prev_ub"] = st, ub
                        # next rep's input loads + Q/K projections go out
                        # after this rep's last bias DMA (no SP queue HOL)
                        if next_head is not None and i == 29:
                            next_head()

                def tail_parts():
                    prev, prev_ub = state["prev"], state["prev_ub"]

                    def part1():
                        emit_pv(prev, prev_ub)
                        emit_combine(prev[0], prev[1])

                    return [part1,
                            lambda: emit_transpose_half(1),
                            lambda: emit_outproj_half(1, tail=True)]

                ctx["head"] = head
                ctx["steps"] = emit_steps
                ctx["tail_parts"] = tail_parts
                return ctx

            # cross-rep software pipeline: the next rep's input DMAs and
            # Q/K projections are emitted before this rep's output tail, so
            # engines stay busy across the rep boundary.
            rctx = [make_rep(first=(r == 0)) for r in range(reps)]
            rctx[0]["head"]()
            deferred = []
            for r in range(reps):
                nh = rctx[r + 1]["head"] if r + 1 < reps else None
                rctx[r]["steps"](deferred, next_head=nh)
                deferred = rctx[r]["tail_parts"]()
            for f in deferred:
                f()
    nc.compile()
    return nc


def _get_kernel(reps=1, zb=True):
    key = f"k{reps}z{int(zb)}"
    if key not in _COMPILED:
        _COMPILED[key] = _build(reps, zb)
    return _COMPILED[key]


def _to_bf16(a):
    import jax.numpy as jnp
    return np.asarray(jnp.asarray(np.asarray(a), dtype=jnp.bfloat16))


def _zero_bias(bq, bk, bv, bp):
    return not (np.any(np.asarray(bq)) or np.any(np.asarray(bk))
                or np.any(np.asarray(bv)) or np.any(np.asarray(bp)))


def _prep_inputs(x_q, x_kv, coords_q, coords_k, alpha_map,
                 Wq, bq, Wk, bk, Wv, bv,
                 lambda_q1, lambda_k1, lambda_q2, lambda_k2,
                 rpe_table, Wp, bp, zb=None):
    if zb is None:
        zb = _zero_bias(bq, bk, bv, bp)
    x_q = np.asarray(x_q, dtype=np.float32)
    x_kv = np.asarray(x_kv, dtype=np.float32)
    coords_q = np.asarray(coords_q)
    coords_k = np.asarray(coords_k)
    alpha_map = np.asarray(alpha_map, dtype=np.float32)
    rpe = np.asarray(rpe_table, dtype=np.float32)
    Wq = np.asarray(Wq, dtype=np.float32)
    Wk = np.asarray(Wk, dtype=np.float32)
    Wv = np.asarray(Wv, dtype=np.float32)
    Wp = np.asarray(Wp, dtype=np.float32)
    bq = np.asarray(bq, dtype=np.float32)
    bk = np.asarray(bk, dtype=np.float32)
    bv = np.asarray(bv, dtype=np.float32)
    bp = np.asarray(bp, dtype=np.float32)

    lam1 = np.exp(np.sum(np.asarray(lambda_q1) * np.asarray(lambda_k1), axis=-1))
    lam2 = np.exp(np.sum(np.asarray(lambda_q2) * np.asarray(lambda_k2), axis=-1))
    lam = (lam1 - lam2 + LAMBDA_INIT).astype(np.float32)  # [4] per pair

    B = x_q.shape[0]
    # per-batch exp(bias) [q, k, H] and transposed bias, computed once
    expb_bT = []
    for b in range(B):
        rel = coords_q[b][:, None, :] - coords_k[b][None, :, :] + MAX_DIST
        rel = np.clip(rel, 0, 2 * MAX_DIST)
        idx = rel[..., 0] * (2 * MAX_DIST + 1) + rel[..., 1]  # [q, k]
        expb_bT.append(np.exp(rpe[idx]).transpose(2, 1, 0))  # [H, k, q]

    in_maps = []
    for c in range(N_CORES):
        b, hg = divmod(c, 2)
        hqk = [2 * hg, 2 * hg + 1, 2 * hg + 4, 2 * hg + 5]
        sl = lambda h: slice(64 * h, 64 * (h + 1))

        wq_l = np.concatenate([Wq.T[:, sl(h)] for h in hqk], 1) * SCALE
        wk_l = np.concatenate([Wk.T[:, sl(h)] for h in hqk], 1)
        # V col order per pair p: [V2 | V1] = heads [2hg+4+p, 2hg+p]
        hv = [2 * hg + 4, 2 * hg, 2 * hg + 5, 2 * hg + 1]
        wv_l = np.concatenate([Wv.T[:, sl(h)] for h in hv], 1)
        # xcat col order per pair p: [x1 | x2] = out dims [2hg+p, 2hg+4+p]
        hx = [2 * hg, 2 * hg + 4, 2 * hg + 1, 2 * hg + 5]
        wp_l = np.concatenate([Wp.T[sl(h), :] for h in hx], 0)

        bq_s = (np.concatenate([bq[sl(h)] for h in hqk]) * SCALE).reshape(2, 128).T
        bk_s = np.concatenate([bk[sl(h)] for h in hqk]).reshape(2, 128).T
        bqk_l = np.concatenate([bq_s, bk_s], 1)  # [128, 4]
        bv_s = np.concatenate([bv[sl(h)] for h in hv])
        bv_l = np.tile(bv_s[None, :], (128, 1))
        bp_l = bp.reshape(4, 128).T if hg == 0 else np.zeros((128, 4), np.float32)

        alpha_r = alpha_map[b, :, 0].reshape(8, 128).T  # [128, qt]
        alp_l = np.concatenate(
            [1.0 + alpha_r, alpha_r * lam[2 * hg], alpha_r * lam[2 * hg + 1]],
            1).reshape(128, 24, 1)

        # bias [pair, qhalf, ktile, k, 2*512]: head j of pair p, transposed
        eT = expb_bT[b]  # [H, k, q]
        bias_l = np.empty((2, 2, 8, 128, 2, 512), np.float32)
        for p in range(2):
            h1, h2 = 2 * hg + p, 2 * hg + 4 + p
            for qh in range(2):
                qs = slice(512 * qh, 512 * (qh + 1))
                bias_l[p, qh, :, :, 0, :] = eT[h1][:, qs].reshape(8, 128, 512)
                bias_l[p, qh, :, :, 1, :] = eT[h2][:, qs].reshape(8, 128, 512)
        bias_l = bias_l.reshape(2, 2, 8, 128, 1024)

        in_maps.append({
            "xq": _to_bf16(np.ascontiguousarray(x_q[b].T).reshape(4, 128, NQ)
                           .transpose(1, 0, 2)),
            "xkv": _to_bf16(np.ascontiguousarray(x_kv[b].T).reshape(4, 128, NKV)
                            .transpose(1, 0, 2)),
            "wq": _to_bf16(wq_l.reshape(4, 128, 256).transpose(1, 0, 2)),
            "wk": _to_bf16(wk_l.reshape(4, 128, 256).transpose(1, 0, 2)),
            "wv": _to_bf16(wv_l.reshape(4, 128, 256).transpose(1, 0, 2)),
            "wp": _to_bf16(wp_l.reshape(2, 128, DIM).transpose(1, 0, 2)),
            "alp": np.ascontiguousarray(alp_l),
            "biasT": _to_bf16(bias_l),
        })
        if not zb:
            in_maps[-1].update({
                "bqk": np.ascontiguousarray(bqk_l),
                "bv": np.ascontiguousarray(bv_l),
                "bp": np.ascontiguousarray(bp_l),
            })
    return in_maps


def kernel(x_q, x_kv, coords_q, coords_k, alpha_map,
           Wq, bq, Wk, bk, Wv, bv,
           lambda_q1, lambda_k1, lambda_q2, lambda_k2,
           rpe_table, Wp, bp):
    from concourse.bass_utils import run_bass_kernel_spmd

    zb = _zero_bias(bq, bk, bv, bp)
    nc = _get_kernel(zb=zb)
    in_maps = _prep_inputs(x_q, x_kv, coords_q, coords_k, alpha_map,
                           Wq, bq, Wk, bk, Wv, bv,
                           lambda_q1, lambda_k1, lambda_q2, lambda_k2,
                           rpe_table, Wp, bp, zb=zb)
    res = run_bass_kernel_spmd(nc, in_maps, list(range(N_CORES)))
    B = np.asarray(x_q).shape[0]
    out = np.zeros((B, NQ, DIM), dtype=np.float32)
    for b in range(B):
        out[b] = (res.results[2 * b]["out_T"].astype(np.float32) +
                  res.results[2 * b + 1]["out_T"].astype(np.float32)).T
    return out

